# revision 83
# baseline (speedup 1.0000x reference)
"""Trainium2 Bass kernel for nn_L3_31799937859925 (sparse_attention).

Strategy (folded algebra + fp8 DoubleRow):
- Queries sorted by label on host; each of 8 cores gets a contiguous 2048-query
  slice (pure data parallel, no collectives). kv rows label-sorted; each
  512-query tile reads a small contiguous kv window W (+ additive mask bias).
- Algebraic folding (exact): rms(up) applies a per-query scalar s_q, so
    Wmix_up @ (w_out*up) * s_q = (Wmix_up @ diag(w_out) @ Wup) @ comb * s_q
  and comb = V^T p, so the whole up+mix_up path collapses to
    (Wfold @ V_win^T) @ (p * s_q/z)  -- a per-tile [1024,W] matrix (host-built).
  ||up||^2 (needed for s_q) = ||(V L)^T p||^2 / z^2 where Wup^T Wup = L L^T
  (Cholesky), giving stats from a [W,512] "VL" matmul + square + column-sum.
- rms_in scalars computed on host (exact, f64); applied as a per-query
  broadcast on the score logits (c_q row folded with the fp8 descales).
- Heavy matmuls are fp8e4(e4m3) DoubleRow (2 k-tiles/instr, 0.5 cycles/row)
  with hi/lo splits: scores and the WV path use 3 terms (Wh*Xh+Wh*Xl+Wl*Xh,
  rel err ~1e-3); the mix-x path keeps the Wl term on only 1 of 4 k-tile
  pairs and the h-stats use the pu hi plane only (total rel err 1.57e-2 vs
  the 2e-2 gate; execution is deterministic so this margin is exact).
  Stat sums (z, sum h^2) are DoubleRow ones-matmuls (h^2 in fp8e5 for range).
- rsqrt computed as Exp(-0.5*Ln(w)); all Activation funcs (Exp/Ln/Copy/Square)
  live in one act-table set, preloaded once -> no per-tile table reloads.
- Software pipeline (PE order): S=scores, T=stats, B=bcast+split, F=fold/mix:
  preamble S0 T0 S1 B0, then F_a(k) T(k+1) F_b(k) B(k+1) S(k+2). B sits right
  after the stats chain completes and before the next exp chain is queued on
  Activation, so push/pusl are always ready when F needs them. Within F the
  x-path matmuls (no stats dependency) run before the attention-path ones.
  PSUM banks: 2 scores + 2 shared (h/broadcast/z/hs rows) + 4 fold outputs.
"""
import numpy as np
import ml_dtypes

import concourse.bass as bass
import concourse.tile as tile
from concourse import bacc, mybir
import concourse.bass_utils as bass_utils

F32 = mybir.dt.float32
F32R = mybir.dt.float32r
BF16 = mybir.dt.bfloat16
F8 = mybir.dt.float8e4
F8E5 = mybir.dt.float8e5
AF = mybir.ActivationFunctionType
MUL = mybir.AluOpType.mult
ADD = mybir.AluOpType.add
SUB = mybir.AluOpType.subtract
DR = mybir.MatmulPerfMode.DoubleRow

NPF8 = ml_dtypes.float8_e4m3
NPF8E5 = ml_dtypes.float8_e5m2
NPBF = ml_dtypes.bfloat16

H, N_EMB, D_EMB, D_UP = 1024, 8192, 512, 2048
B, T = 4, 4096
BT = B * T                  # 16384
NC = 8                      # cores
NQ = BT // NC               # 2048 queries per core
QT = 512                    # queries per q-tile
NQT = NQ // QT              # 4 q-tiles per core
HC = H // 128               # 8
XP = HC // 2                # 4 hc DoubleRow pairs
DC = D_EMB // 128           # 4
MC = H // 128               # 8 output chunks

# fp8 scales (products must match so fold+mixx share PSUM accumulation)
SX = 8.0                    # raw x
SW = 512.0                  # Wx
SXN = 8.0                   # normalized x
SK = 512.0                  # kt
SWV = 4096.0                # WV (pu_scaled carries no extra scale)
SVL = 512.0                 # VL
SH = 64.0                   # h^2 computed as (h_true*SH)^2
DESCALE = 1.0 / 4096.0      # 1/(SX*SW) == 1/(SXN*SK) == 1/SWV

LAST_RESULTS = None         # BassKernelResults of the most recent run (for test.py)
LAST_EXEC_S = None
_PROGRAM_CACHE = {}


def _build_program(W):
    """SPMD single-core program. W = kv window width (multiple of 128)."""
    n_kvc = W // 128
    n_kvp = n_kvc // 2          # full kv DoubleRow pairs
    kv_odd = n_kvc % 2
    nc = bacc.Bacc("TRN2", target_bir_lowering=False, debug=False,
                   enable_asserts=False)

    kth_in = nc.dram_tensor("kth_in", [NQT, 128, HC, W], F8, kind="ExternalInput")
    ktl_in = nc.dram_tensor("ktl_in", [NQT, 128, HC, W], F8, kind="ExternalInput")
    xh_in = nc.dram_tensor("xh_in", [128, HC, NQ], F8, kind="ExternalInput")
    xl_in = nc.dram_tensor("xl_in", [128, HC, NQ], F8, kind="ExternalInput")
    crow_in = nc.dram_tensor("crow_in", [1, NQ], F32R, kind="ExternalInput")
    vlh_in = nc.dram_tensor("vlh_in", [NQT, 128, n_kvc, D_EMB], F8, kind="ExternalInput")
    wvp_in = nc.dram_tensor("wvp_in", [NQT, 128, 2, n_kvc, H], F8, kind="ExternalInput")
    b_in = nc.dram_tensor("b_in", [NQT, 128, n_kvc, QT], F8E5, kind="ExternalInput")
    wxp_in = nc.dram_tensor("wxp_in", [128, MC, XP + 1, 2, 128], F8, kind="ExternalInput")
    out_d = nc.dram_tensor("out_d", [MC, 128, NQ], BF16, kind="ExternalOutput")

    from contextlib import ExitStack
    with tile.TileContext(nc) as tc, ExitStack() as ctx:
        ec = ctx.enter_context
        cst = ec(tc.tile_pool(name="cst", bufs=1))
        pkt = ec(tc.tile_pool(name="pkt", bufs=3))
        pwx = ec(tc.tile_pool(name="pwx", bufs=1))
        pxr = ec(tc.tile_pool(name="pxr", bufs=3))
        pwv = ec(tc.tile_pool(name="pwv", bufs=3))
        pvl = ec(tc.tile_pool(name="pvl", bufs=3))
        pb = ec(tc.tile_pool(name="pb", bufs=3))
        ppu = ec(tc.tile_pool(name="ppu", bufs=3))
        ppb = ec(tc.tile_pool(name="ppb", bufs=3))
        pt = ec(tc.tile_pool(name="pt", bufs=4))
        phs = ec(tc.tile_pool(name="phs", bufs=3))
        prr = ec(tc.tile_pool(name="prr", bufs=8))
        prr2 = ec(tc.tile_pool(name="prr2", bufs=2))
        psb = ec(tc.tile_pool(name="psb", bufs=3))
        pps = ec(tc.tile_pool(name="pps", bufs=4))
        pph = ec(tc.tile_pool(name="pph", bufs=3))
        ppl = ec(tc.tile_pool(name="ppl", bufs=3))
        po = ec(tc.tile_pool(name="po", bufs=6))
        # PSUM: 2+2+4 = 8 banks (bcasts and z/hs rows borrow from ph)
        psc = ec(tc.tile_pool(name="psc", bufs=2, space="PSUM"))
        ph = ec(tc.tile_pool(name="ph", bufs=2, space="PSUM"))
        pout = ec(tc.tile_pool(name="pout", bufs=4, space="PSUM"))

        # DoubleRow ldweights needs the k-tile-pair stride 16B-aligned, so the
        # ones vectors are padded to 16 columns and sliced.
        ones8f = cst.tile([128, 2, 16], F32)
        nc.vector.memset(ones8f, 1.0)
        ones8 = cst.tile([128, 2, 16], F8)
        nc.vector.tensor_copy(ones8, ones8f)
        ones85 = cst.tile([128, 2, 16], F8E5)
        nc.vector.tensor_copy(ones85, ones8f)
        ones_rf = cst.tile([1, 128], F32)
        nc.vector.memset(ones_rf, 1.0)
        ones_row_r = cst.tile([1, 128], F32R)
        nc.vector.tensor_copy(ones_row_r, ones_rf)

        # Preload the one act-func table containing every function we use, so
        # the compiler's table-load pass finds it loaded on all paths and
        # never reloads (each InstLoadActFuncSet costs ~1.3us on Activation).
        try:
            from concourse.hw_specs import get_activation_tables
            _tabs = list(get_activation_tables(nc.m.arch).items())
            _need = {AF.Exp, AF.Ln, AF.Copy, AF.Square}
            _set_id = next(i for i, (_n, _s) in enumerate(_tabs)
                           if _need <= _s)
            nc.scalar.add_instruction(mybir.InstLoadActFuncSet(
                name=nc.scalar.bass.get_next_instruction_name(),
                act_func_set_id=_set_id, ins=[], outs=[]))
        except Exception:
            pass  # compiler inserts its own (slower) table loads

        wxp_t = pwx.tile([128, MC, XP + 1, 2, 128], F8)
        crow_t = cst.tile([1, NQ], F32R)
        nc.sync.dma_start(crow_t[:], crow_in.ap())

        def emit_B(st):
            """Broadcast s_q/z and split pu_scaled into fp8 hi/lo."""
            (qt, srow, pu_f, push, pusl) = st
            sb_ps = ph.tile([128, QT], F32, tag="h")
            nc.tensor.matmul(sb_ps, lhsT=ones_row_r, rhs=srow,
                             start=True, stop=True)
            for kvc in range(n_kvc):
                pus_f = pps.tile([128, QT], F32, tag="pus")
                nc.vector.tensor_tensor(pus_f, pu_f[:, kvc, :], sb_ps, MUL)
                nc.scalar.activation(push[:, kvc, :], pus_f, AF.Copy)
                nc.vector.tensor_tensor(pusl[:, kvc, :], pus_f,
                                        push[:, kvc, :], SUB)

        def emit_F(st, mcs):
            (qt, push, pusl, wvh_t, wvl_t, xh_t, xl_t) = st
            qs = slice(qt * QT, (qt + 1) * QT)
            for mc in mcs:
                o_ps = pout.tile([128, QT], F32, tag="out")
                ms = slice(mc * 128, (mc + 1) * 128)
                fold_ops = []
                # x-path first: depends only on x/wx (ready early)
                for bx in (xh_t, xl_t):
                    for p in range(XP):
                        fold_ops.append((wxp_t[:, mc, p],
                                         bx[:, 2 * p:2 * p + 2, :], DR))
                # lo-weight correction, k-tile pair 0 only (error headroom)
                fold_ops.append((wxp_t[:, mc, XP], xh_t[:, 0:2, :], DR))
                for aw, bp in ((wvh_t, push), (wvh_t, pusl), (wvl_t, push)):
                    for pp in range(n_kvp):
                        fold_ops.append((aw[:, 2 * pp:2 * pp + 2, ms],
                                         bp[:, 2 * pp:2 * pp + 2, :], DR))
                    if kv_odd:
                        fold_ops.append((aw[:, n_kvc - 1, ms],
                                         bp[:, n_kvc - 1, :], None))
                for i, (lhsT, rhs, pm) in enumerate(fold_ops):
                    nc.tensor.matmul(o_ps, lhsT=lhsT, rhs=rhs,
                                     start=(i == 0), stop=(i == len(fold_ops) - 1),
                                     perf_mode=pm)
                o_sb = po.tile([128, QT], BF16, tag="o")
                if mc % 2 == 0:
                    nc.scalar.activation(o_sb, o_ps, AF.Copy, scale=DESCALE)
                else:
                    nc.vector.tensor_scalar_mul(o_sb, o_ps, DESCALE)
                nc.sync.dma_start(out_d.ap()[mc][:, qs], o_sb[:])

        def emit_S_dma(qt, eng=None):
            """Critical DMAs for tile qt (scores inputs)."""
            eng = eng or nc.sync
            qs = slice(qt * QT, (qt + 1) * QT)
            kth_t = pkt.tile([128, HC, W], F8, tag="kth")
            eng.dma_start(kth_t[:], kth_in.ap()[qt])
            xh_t = pxr.tile([128, HC, QT], F8, tag="xh")
            if qt < 2:
                # split so the first score pairs can start one half earlier
                eng.dma_start(xh_t[:, 0:HC // 2, :],
                              xh_in.ap()[:, 0:HC // 2, qs])
                eng.dma_start(xh_t[:, HC // 2:, :],
                              xh_in.ap()[:, HC // 2:, qs])
            else:
                eng.dma_start(xh_t[:], xh_in.ap()[:, :, qs])
            ktl_t = pkt.tile([128, HC, W], F8, tag="ktl")
            eng.dma_start(ktl_t[:], ktl_in.ap()[qt])
            xl_t = pxr.tile([128, HC, QT], F8, tag="xl")
            eng.dma_start(xl_t[:], xl_in.ap()[:, :, qs])
            b_t = pb.tile([128, n_kvc, QT], F8E5, tag="b")
            eng.dma_start(b_t[:], b_in.ap()[qt])
            return (qt, qs, xh_t, xl_t, kth_t, ktl_t, b_t)

        def emit_wx():
            # stream Wx per output chunk so F(0) can start on chunk 0
            # while the rest is still in flight
            for mc in range(MC):
                nc.sync.dma_start(wxp_t[:, mc], wxp_in.ap()[:, mc])

        def emit_S(dma_st):
            """Scores matmuls + logit scale/mask + exp + pub split."""
            (qt, qs, xh_t, xl_t, kth_t, ktl_t, b_t) = dma_st

            # c_q/(SX*SK) broadcast for this tile's logits
            cb_ps = ph.tile([128, QT], F32, tag="h")
            nc.tensor.matmul(cb_ps, lhsT=ones_row_r,
                             rhs=crow_t[:, qs],
                             start=True, stop=True)
            cb_sb = psb.tile([128, QT], F32, tag="cb")
            nc.scalar.activation(cb_sb, cb_ps, AF.Copy)

            # scores (fp8 DoubleRow, 3-term) -> logits -> exp
            pu_f = ppu.tile([128, n_kvc, QT], F32, tag="puf")
            pubh = ppb.tile([128, n_kvc, QT], F8, tag="pubh")
            publ = ppb.tile([128, n_kvc, QT], F8, tag="publ")
            for kvc in range(n_kvc):
                s_ps = psc.tile([128, QT], F32, tag="sc")
                kvs = slice(kvc * 128, kvc * 128 + 128)
                n_ops = 3 * XP
                i = 0
                for a, bb_ in ((kth_t, xh_t), (ktl_t, xh_t), (kth_t, xl_t)):
                    for p in range(XP):
                        nc.tensor.matmul(
                            s_ps, lhsT=a[:, 2 * p:2 * p + 2, kvs],
                            rhs=bb_[:, 2 * p:2 * p + 2, :],
                            start=(i == 0), stop=(i == n_ops - 1), perf_mode=DR)
                        i += 1
                tm_t = pt.tile([128, QT], F32, tag="tm")
                nc.vector.tensor_tensor(tm_t, s_ps, cb_sb, MUL)
                t_t = pt.tile([128, QT], F32, tag="t")
                nc.vector.tensor_tensor(t_t, tm_t, b_t[:, kvc, :], ADD)
                nc.scalar.activation(pu_f[:, kvc, :], t_t, AF.Exp)
                nc.scalar.activation(pubh[:, kvc, :], pu_f[:, kvc, :], AF.Copy)
                nc.vector.tensor_tensor(publ[:, kvc, :], pu_f[:, kvc, :],
                                        pubh[:, kvc, :], SUB)
            return (qt, pu_f, pubh, publ, xh_t, xl_t)

        def emit_aux(qt):
            """vl/wv DMAs for tile qt (issued later than the S-critical set
            so the fill window prioritizes score inputs)."""
            vlh_t = pvl.tile([128, n_kvc, D_EMB], F8, tag="vlh")
            nc.sync.dma_start(vlh_t[:], vlh_in.ap()[qt])
            wvp_t = pwv.tile([128, 2, n_kvc, H], F8, tag="wv")
            nc.sync.dma_start(wvp_t[:], wvp_in.ap()[qt])
            return (vlh_t, wvp_t[:, 0], wvp_t[:, 1])

        def emit_T(stS, aux):
            """Stat matmuls (z, h, sum h^2) + the s_q/z scalar row chain."""
            (qt, pu_f, pubh, publ, xh_t, xl_t) = stS
            (vlh_t, wvh_t, wvl_t) = aux
            # h uses only the pu hi plane (||up||^2 tolerates ~2.6% per-h
            # error: it averages over 512 dims and feeds a rsqrt); z keeps
            # hi+lo since it scales comb directly. h first: it needs only
            # pubh, which is ready one chain-hop before publ.
            zrow_ps = ph.tile([1, QT], F32, tag="h")
            hrow_ps = ph.tile([1, QT], F32, tag="h")
            hsq = phs.tile([128, DC, QT], F8E5, tag="hsq")
            for jc in range(DC):
                h_ps = ph.tile([128, QT], F32, tag="h")
                js = slice(jc * 128, (jc + 1) * 128)
                hops = []
                for pp in range(n_kvp):
                    hops.append((vlh_t[:, 2 * pp:2 * pp + 2, js],
                                 pubh[:, 2 * pp:2 * pp + 2, :], DR))
                if kv_odd:
                    hops.append((vlh_t[:, n_kvc - 1, js],
                                 pubh[:, n_kvc - 1, :], None))
                for i, (lhsT, rhs, pm) in enumerate(hops):
                    nc.tensor.matmul(h_ps, lhsT=lhsT, rhs=rhs,
                                     start=(i == 0), stop=(i == len(hops) - 1),
                                     perf_mode=pm)
                nc.scalar.activation(hsq[:, jc, :], h_ps, AF.Square,
                                     scale=SH / SVL)
            zops = []
            for pb8 in (pubh, publ):
                for pp in range(n_kvp):
                    zops.append((ones8[:, :, 0:1],
                                 pb8[:, 2 * pp:2 * pp + 2, :], DR))
                if kv_odd:
                    zops.append((ones8[:, 0, 0:1], pb8[:, n_kvc - 1, :], None))
            for i, (lhsT, rhs, pm) in enumerate(zops):
                nc.tensor.matmul(zrow_ps, lhsT=lhsT, rhs=rhs,
                                 start=(i == 0), stop=(i == len(zops) - 1),
                                 perf_mode=pm)
            for c in range(DC // 2):
                nc.tensor.matmul(hrow_ps, lhsT=ones85[:, :, 0:1],
                                 rhs=hsq[:, 2 * c:2 * c + 2, :],
                                 start=(c == 0), stop=(c == DC // 2 - 1),
                                 perf_mode=DR)

            # ---- s_q/z = rsqrt(hs/(D_UP*SH^2) + 1e-6*z^2) via Exp(-0.5 Ln w)
            z2s = prr.tile([1, QT], F32, tag="rr")
            nc.scalar.activation(z2s, zrow_ps, AF.Square, scale=1e-3)
            wrow = prr.tile([1, QT], F32, tag="rr")
            nc.vector.scalar_tensor_tensor(wrow, hrow_ps,
                                           1.0 / (D_UP * SH * SH),
                                           z2s, MUL, ADD)
            lrow = prr.tile([1, QT], F32, tag="rr")
            nc.scalar.activation(lrow, wrow, AF.Ln)
            srow = prr2.tile([1, QT], F32R, tag="rr2")
            nc.scalar.activation(srow, lrow, AF.Exp, scale=-0.5)

            push = pph.tile([128, n_kvc, QT], F8, tag="push")
            pusl = ppl.tile([128, n_kvc, QT], F8, tag="pusl")
            stB = (qt, srow, pu_f, push, pusl)
            stF = (qt, push, pusl, wvh_t, wvl_t, xh_t, xl_t)
            return stB, stF

        # 2-deep software pipeline; steady-state PE order per iteration is
        # F(k) T(k+1) S(k+2) B(k+1). B sits at iteration END so its split ops
        # are queued on Act/DVE ahead of the next iteration's exp chain,
        # making push/pusl(k+1) ready long before F(k+1) needs them. The
        # vl/wv DMAs ride with T, Wx streams per-chunk inside B(0), so the
        # fill window prioritizes the score inputs.
        MCA = list(range(4))            # fold chunks before T (pair-aligned)
        MCB = list(range(4, MC))        # fold chunks between T and B
        stS = [None] * NQT
        stTB = [None] * NQT
        aux = [None] * NQT
        d0 = emit_S_dma(0)
        stS[0] = emit_S(d0)
        d1 = emit_S_dma(1)
        aux[0] = emit_aux(0)
        stTB[0] = emit_T(stS[0], aux[0])
        emit_wx()
        stS[1] = emit_S(d1)
        emit_B(stTB[0][0])
        for k in range(NQT):
            if k + 1 < NQT:
                emit_F(stTB[k][1], MCA)
                aux[k + 1] = emit_aux(k + 1)
                d_next = emit_S_dma(k + 2) if k + 2 < NQT else None
                stTB[k + 1] = emit_T(stS[k + 1], aux[k + 1])
                emit_F(stTB[k][1], MCB)
                emit_B(stTB[k + 1][0])
                if d_next is not None:
                    stS[k + 2] = emit_S(d_next)
            else:
                emit_F(stTB[k][1], list(range(MC)))

    nc.compile()
    return nc


def _get_program(W):
    if W not in _PROGRAM_CACHE:
        _PROGRAM_CACHE[W] = _build_program(W)
    return _PROGRAM_CACHE[W]


def _split8(a):
    hi = a.astype(NPF8)
    lo = (a - hi.astype(np.float32)).astype(NPF8)
    return hi, lo


def kernel(**inputs) -> np.ndarray:
    global LAST_RESULTS, LAST_EXEC_S
    inp = np.asarray(inputs["input"], np.float32)
    fw = np.asarray(inputs["fw"]).astype(np.int64)
    seq_sort = np.asarray(inputs["seq_sort"]).astype(np.int64)
    keep_cols = np.asarray(inputs["keep_cols"]).astype(np.int64)
    emb_alloc = np.asarray(inputs["emb_alloc"]).astype(np.int64)
    starts = np.asarray(inputs["starts"]).astype(np.int64)
    ends = np.asarray(inputs["ends"]).astype(np.int64)
    bb = int(np.asarray(inputs["bb"]))
    w_k = np.asarray(inputs["w_k_weight"], np.float32)
    w_v = np.asarray(inputs["w_v_weight"], np.float32)
    w_up = np.asarray(inputs["w_up_weight"], np.float32)
    w_mix = np.asarray(inputs["w_mix_weight"], np.float32)
    w_in = np.asarray(inputs["norm_in_weight"], np.float32)
    w_out = np.asarray(inputs["norm_out_weight"], np.float32)

    x = inp.reshape(BT, H)
    nb = BT // bb
    st = starts.reshape(nb, bb).min(axis=1)
    en = ends.reshape(nb, bb).max(axis=1)

    # sort queries by label (stable); sorted row s holds query fw[order[s]]
    order = np.argsort(seq_sort, kind="stable")
    perm = fw[order]
    lab_q = seq_sort[order]
    blk_q = order // bb
    st_q = st[blk_q]
    en_q = en[blk_q]
    x_sorted = x[perm]                       # [BT, H]

    # kv side: keep + label-sort; fold norm_in into K
    la = emb_alloc[keep_cols]
    M = la.shape[0]
    kv_order = np.argsort(la, kind="stable")
    la_s = la[kv_order]
    kvpos = kv_order
    Bm = (w_k[keep_cols] * w_in[None, :])[kv_order]   # [M, H]
    Cm = w_v[keep_cols][kv_order]            # [M, D_EMB]

    counts = np.bincount(la_s, minlength=64)
    gstart = np.concatenate([[0], np.cumsum(counts)])

    NT = BT // QT                            # 32 global q-tiles
    win = np.empty(NT, np.int64)
    need = 0
    for g in range(NT):
        l0 = lab_q[g * QT]
        l1 = lab_q[(g + 1) * QT - 1]
        win[g] = gstart[l0]
        need = max(need, gstart[l1 + 1] - gstart[l0])
    W = max(256, int(-(-need // 128) * 128))
    n_kvc = W // 128

    Mp = M + W
    Bm_p = np.zeros((Mp, H), np.float32); Bm_p[:M] = Bm
    Cm_p = np.zeros((Mp, D_EMB), np.float32); Cm_p[:M] = Cm
    la_p = np.full(Mp, -1, np.int64); la_p[:M] = la_s
    kvpos_p = np.full(Mp, -1, np.int64); kvpos_p[:M] = kvpos

    # mask bias per (sorted row, window col)
    kvi = win[:, None] + np.arange(W)[None, :]           # [NT, W]
    la_w = la_p[kvi]
    kp_w = kvpos_p[kvi]
    lab_t = lab_q.reshape(NT, QT)
    st_t = st_q.reshape(NT, QT)
    en_t = en_q.reshape(NT, QT)
    valid = ((la_w[:, None, :] == lab_t[:, :, None])
             & (kp_w[:, None, :] >= st_t[:, :, None])
             & (kp_w[:, None, :] < en_t[:, :, None]))    # [NT, QT, W]
    # -30000 fits fp8e5 (max 57344); exp(-30000) == 0 in f32
    bias = np.where(valid, np.float32(0), np.float32(-30000.0))

    # folded weights
    wf = w_mix[:, :D_UP] * w_out[None, :]                # [H, D_UP]
    Wfold = wf @ w_up                                    # [H, D_EMB]
    Wx = w_mix[:, D_UP:]                                 # [H, H]
    G = w_up.T.astype(np.float64) @ w_up.astype(np.float64)
    L = np.linalg.cholesky(G + 1e-12 * np.eye(D_EMB)).astype(np.float32)
    VL = (Cm_p @ L) * SVL                                # [Mp, D_EMB] scaled

    # rms_in scalars (host, f64); folded descales go into the c_q row that
    # scales the raw-x score psum on device
    xs64 = x_sorted.astype(np.float64)
    c_q = 1.0 / np.sqrt((xs64 ** 2).mean(axis=1) + 1e-6)
    crow = (c_q / (SX * SK)).astype(np.float32)          # [BT]

    xh_f, xl_f = _split8(x_sorted * SX)                  # [BT, H] fp8
    wxs = Wx * SW
    # wx[k, mc, p, i, m] = Wxs[mc*128+m, (2p+i)*128+k]
    wx_r = wxs.reshape(MC, 128, XP, 2, 128).transpose(4, 0, 2, 3, 1)
    wxh = wx_r.astype(NPF8)
    wxl0 = (wx_r[:, :, 0:1] - wxh[:, :, 0:1].astype(np.float32)).astype(NPF8)
    wxp = np.ascontiguousarray(np.concatenate([wxh, wxl0], axis=2))

    def to_core_x(a8):
        # [rows, H] fp8 -> [128, HC, NQ]
        return np.ascontiguousarray(
            a8.T.reshape(HC, 128, -1).transpose(1, 0, 2))

    KT_full = np.ascontiguousarray(Bm_p.T) * SK          # [H, Mp] f32

    in_maps = []
    for c in range(NC):
        rows = slice(c * NQ, (c + 1) * NQ)
        kt_c = np.empty((NQT, 128, HC, W), np.float32)
        vl_c = np.empty((NQT, 128, n_kvc, D_EMB), np.float32)
        wvh_c = np.empty((NQT, 128, n_kvc, H), NPF8)
        wvl_c = np.empty((NQT, 128, n_kvc, H), NPF8)
        b_c = np.empty((NQT, 128, n_kvc, QT), NPF8E5)
        for qt in range(NQT):
            g = c * NQT + qt
            w0 = win[g]
            ws = slice(w0, w0 + W)
            kt_c[qt] = KT_full[:, ws].reshape(HC, 128, W).transpose(1, 0, 2)
            vl_c[qt] = VL[ws].reshape(n_kvc, 128, D_EMB).transpose(1, 0, 2)
            WV = (Wfold @ Cm_p[ws].T) * SWV              # [H, W]
            WVt = WV.T.reshape(n_kvc, 128, H).transpose(1, 0, 2)
            wvh_q = WVt.astype(NPF8)
            wvh_c[qt] = wvh_q
            wvl_c[qt] = (WVt - wvh_q.astype(np.float32)).astype(NPF8)
            b_c[qt] = bias[g].T.reshape(n_kvc, 128, QT).transpose(1, 0, 2)
        kth_c = kt_c.astype(NPF8)
        ktl_c = (kt_c - kth_c.astype(np.float32)).astype(NPF8)
        vlh_c = vl_c.astype(NPF8)
        wvp_c = np.ascontiguousarray(np.stack([wvh_c, wvl_c], axis=2))
        in_maps.append({
            "kth_in": kth_c, "ktl_in": ktl_c,
            "xh_in": to_core_x(xh_f[rows]), "xl_in": to_core_x(xl_f[rows]),
            "crow_in": np.ascontiguousarray(crow[rows][None, :]),
            "vlh_in": vlh_c,
            "wvp_in": wvp_c, "b_in": b_c,
            "wxp_in": wxp,
        })

    prog = _get_program(W)
    import time as _time
    _t0 = _time.time()
    LAST_RESULTS = bass_utils.run_bass_kernel_spmd(prog, in_maps,
                                                   core_ids=list(range(NC)))
    LAST_EXEC_S = _time.time() - _t0
    out_sorted = np.concatenate(
        [np.asarray(r["out_d"], dtype=np.float32).transpose(2, 0, 1).reshape(NQ, H)
         for r in LAST_RESULTS.results],
        axis=0)                                          # [BT, H]
    final = np.empty((BT, H), np.float32)
    final[perm] = out_sorted
    return final.reshape(B, T, H)
